# revision 29
# baseline (speedup 1.0000x reference)
"""Trainium2 Bass kernel for nn_BiochemicalDiffusion.

Computes  out = F - B*x - r * rowsum(x * (A @ x))  for A:[10000,10000] f32,
x:[10000,64] f32, across 8 NeuronCores.

Sharding (all done host-side in this file):
  - A is sharded row-wise: core c gets rows [c*1250, (c+1)*1250).
  - The shard is passed pre-transposed (A_shard^T, [10000, 1250]) so the PE
    can contract over k directly: Ax_shard = A_shard^T.T @ x.
  - x is passed in full to every core (it is tiny), pre-tiled into the
    [128, 79*64] SBUF layout the matmul consumes.
  - Each core computes its [1250, 64] slice of the output; the host
    concatenates them.

A is quantized host-side to fp8e4m3 (1 byte/elem; quantization errors are
zero-mean and average out over the 10000-long contraction -- measured
absmax-rel ~1.3e-3 vs the 2e-2 gate).  Matmuls run in DoubleRow perf mode:
both operands carry a pair of k-tiles ([128, 2, w] APs, contraction 256
per instruction).  PSUM chunk width 250 keeps a DR matmul at 1 column/
cycle ([128,2,500] drops to 2 cycles/col -- measured) and one f32 PSUM
bank per accumulator.

Scheduling notes (from ~14 traced structural experiments):
  - DMA completion semaphores fire up to ~8 us after their data lands
    mid-stream, and DMA issues gate on an 8-lane semaphore recycle
    (issue #k waits for #(k-8)'s sem).  The junk-heat delay line below
    absorbs that jitter; every attempt to remove it exposed the stalls
    directly on the in-order PE and lost 3-10 us.
  - PSUM accumulation groups must not share a PSUM bank.

Everything is hardcoded to the problem shapes; kernel.py is self-contained.
"""

import numpy as np

N = 10000
DIM = 64
NCORES = 8
MSHARD = N // NCORES  # 1250 rows of A / out per core
MT = 125              # m-tile (PSUM partition) size
NMT = MSHARD // MT    # 10 m-tiles per core
KT = 128              # k-tile (contraction) size

F_CONST = 1.0
B_CONST = 0.1
R_CONST = 0.01

NKT2 = 79                 # k-tiles (tile 78 is 16 real rows)
NPAIR = 39                # DoubleRow pairs (tiles 0..77)
KLAST = 16                # real rows in tile 78
KPAD2 = NKT2 * KT
# PSUM chunks: five 250-wide ([128,2,250] DR matmul = 1 column/cycle)
MCH5 = [(i * 250, (i + 1) * 250) for i in range(5)]
# DMA groups alternate between the sync and scalar HWDGE rings; each SDMA
# engine hides one ring's completion latency behind the other ring's data.
KQ8 = 8                   # max k-tiles per DMA group (1.28 MB transfers)
KGROUPS_F8 = ([(0, 2), (2, 2), (4, 4)]
              + [(8 + 8 * i, 8) for i in range(8)]
              + [(72, 4), (76, 2), (78, 1)])
NG8 = len(KGROUPS_F8)     # 14 groups covering all 79 tiles

_nc_cache = {}


def _body_f8(ctx, tc, a_t, xt8_d, xs_d, xst_d, id_d, out_d):
    """Pure-fp8 single pass with DoubleRow (contraction 256 per matmul).

    Baseline schedule (measured best of the structural variants): x k-tile
    pairs stationary, A^T slabs stream in groups alternating between the
    sync and scalar HWDGE rings; junk heat matmuls form a ~17 us delay
    line on the in-order PE that absorbs the DMA completion-semaphore
    jitter and keeps the PE clock-gate (HAM) at 2.4 GHz.

    Deltas vs the original baseline, all tail-local:
      - late per-group heat trimmed (gi >= 5): by then the real stream
        keeps the PE warm, and trailing junk only pushed the final
        matmuls + epilogue past the stream end.
      - epilogue constants in bf16 (240 KB vs 656 KB: land earlier,
        +~3e-4 rel err vs the 2e-2 gate).
      - epilogue: bf16 transposes (single-pass vs fp32 LOW_HIGH) and the
        two affine steps on the scalar (Activation) engine, pipelining
        the per-m-tile chain across PE/DVE/ACT."""
    import concourse.bass  # noqa: F401
    from concourse import mybir

    nc = tc.nc
    f32 = mybir.dt.float32
    fp8 = mybir.dt.float8e4
    bf16 = mybir.dt.bfloat16
    dr = mybir.MatmulPerfMode.DoubleRow

    consts = ctx.enter_context(tc.tile_pool(name="consts", bufs=1))
    slabs = ctx.enter_context(tc.tile_pool(name="slabs", bufs=6))
    psums = ctx.enter_context(tc.tile_pool(name="psums", bufs=1, space="PSUM"))
    ptp = ctx.enter_context(tc.tile_pool(name="ptp", bufs=2, space="PSUM"))
    epil = ctx.enter_context(tc.tile_pool(name="epil", bufs=2))

    # stationary x rides the scalar HWDGE ring ahead of the odd slab
    # groups; the epilogue-only constants are appended at the end of the
    # scalar program.
    xt8 = consts.tile([KT, NKT2, DIM], fp8)
    nc.scalar.dma_start(out=xt8[:, :8, :], in_=xt8_d[:, :8 * DIM])

    accs = [psums.tile([DIM, c1 - c0], f32, name=f"acc{i}", tag=f"acc{i}")
            for i, (c0, c1) in enumerate(MCH5)]

    # PE heat management: the HAM clock gate runs the PE at 1.2 GHz unless
    # it sees sustained busy (~3.4 us windows).  The junk matmuls (no DMA
    # deps; WAW-chained on one scratch tile) get hoisted by the scheduler
    # into a contiguous blob that delays the real stream just long enough
    # to ride out the early completion-sem stalls.
    junk_l = consts.tile([KT, 2, DIM], fp8)
    nc.vector.memset(junk_l, 0.5)
    junk_r = consts.tile([KT, 2, 250], fp8)
    nc.vector.memset(junk_r, 0.5)
    warm = psums.tile([DIM, 250], f32)

    def heat(n):
        for _ in range(n):
            nc.tensor.matmul(warm, lhsT=junk_l, rhs=junk_r,
                             start=True, stop=True, perf_mode=dr)

    heat(16)

    for gi, (k0, g) in enumerate(KGROUPS_F8):
        # alternate groups across the two HWDGE rings so each SDMA engine
        # hides one ring's completion latency behind the other's data
        dma_eng = nc.sync if gi % 2 == 0 else nc.scalar
        slab = slabs.tile([KT, KQ8, MSHARD], fp8, name=f"slab{gi}", tag="slab")
        if g == 1:  # trailing 16-row tile: only partitions 0:15 carry data
            dma_eng.dma_start(out=slab[:KLAST, :1, :],
                              in_=a_t[gi * KT:gi * KT + KLAST, :MSHARD])
            for i, (c0, c1) in enumerate(MCH5):
                nc.tensor.matmul(
                    accs[i],
                    lhsT=xt8[:KLAST, k0, :],
                    rhs=slab[:KLAST, 0, c0:c1],
                    start=False,
                    stop=True,
                )
            continue
        dma_eng.dma_start(out=slab[:, :g, :],
                          in_=a_t[gi * KT:(gi + 1) * KT, :g * MSHARD])
        if gi == 1:
            # the rest of the stationary x rides the scalar ring BEHIND
            # group 1 (whose data the stream needs first); tiles 8+ are
            # not consumed until pair 4, by which time this has landed
            nc.scalar.dma_start(out=xt8[:, 8:, :], in_=xt8_d[:, 8 * DIM:])
        for sub in range(0, g, 2):
            j = (k0 + sub) // 2  # pair index
            lhsT = xt8[:, k0 + sub:k0 + sub + 2, :]
            for i, (c0, c1) in enumerate(MCH5):
                nc.tensor.matmul(
                    accs[i],
                    lhsT=lhsT,
                    rhs=slab[:, sub:sub + 2, c0:c1],
                    start=(j == 0),
                    stop=False,
                    perf_mode=dr,
                )
        # per-group heat exactly as the measured-best baseline: the junk
        # delay line must cover the whole stream (trimming it after gi=4
        # exposed late-group sem stalls and lost ~5 us)
        heat(8 if gi <= 10 else 3)

    # epilogue-only constants (bf16), appended behind the odd slab stream
    xs = consts.tile([MT, NMT * DIM], bf16)
    nc.scalar.dma_start(out=xs, in_=xs_d)
    xst = consts.tile([DIM, MSHARD], bf16)
    nc.scalar.dma_start(out=xst, in_=xst_d)
    ident = consts.tile([DIM, DIM], bf16)
    nc.scalar.dma_start(out=ident, in_=id_d)

    # P = x^T * Ax^T  (elementwise), [64, 1250] bf16 in SBUF (bf16
    # transposes run single-pass; fp32 runs LOW_HIGH two-pass)
    p_full = epil.tile([DIM, MSHARD], bf16, bufs=1)
    for i, (c0, c1) in enumerate(MCH5):
        nc.vector.tensor_mul(p_full[:, c0:c1], xst[:, c0:c1], accs[i])

    # out staged m-tile-major ([p, mt, d]); host restores row order.
    o_full = epil.tile([MT, NMT, DIM], f32, bufs=1)
    for mt in range(NMT):
        pt = ptp.tile([MT, DIM], bf16, name=f"pt{mt}", tag="pt")
        nc.tensor.transpose(
            out=pt, in_=p_full[:, mt * MT:(mt + 1) * MT], identity=ident,
        )
        s = epil.tile([MT, 1], f32, name=f"s{mt}", tag="s")
        nc.vector.tensor_reduce(
            out=s, in_=pt, axis=mybir.AxisListType.X, op=mybir.AluOpType.add,
        )
        t_col = epil.tile([MT, 1], f32, name=f"t{mt}", tag="t")
        # t = s * (-r) + F  and  o = x * (-b) + t on the Activation
        # engine -- the per-mt chain pipelines across PE/DVE/ACT
        nc.scalar.activation(
            out=t_col, in_=s, func=mybir.ActivationFunctionType.Identity,
            bias=F_CONST, scale=-R_CONST,
        )
        nc.scalar.activation(
            out=o_full[:, mt, :], in_=xs[:, mt * DIM:(mt + 1) * DIM],
            func=mybir.ActivationFunctionType.Identity,
            bias=t_col, scale=-B_CONST,
        )
        if mt == 3:
            # output leaves in three waves overlapping the epilogue chain;
            # the last wave is small so its transfer+receipt tail is short
            nc.scalar.dma_start(out=out_d[:, :4 * DIM], in_=o_full[:, :4, :])
        elif mt == 7:
            nc.sync.dma_start(out=out_d[:, 4 * DIM:8 * DIM],
                              in_=o_full[:, 4:8, :])
    nc.scalar.dma_start(out=out_d[:, 8 * DIM:], in_=o_full[:, 8:, :])


def build(layout=None, mm_dtype=None):
    key = "f8"
    if key in _nc_cache:
        return _nc_cache[key]

    from contextlib import ExitStack
    import concourse.tile as tile
    from concourse import bacc
    from concourse import mybir

    f32 = mybir.dt.float32
    bf16 = mybir.dt.bfloat16

    nc = bacc.Bacc(
        "TRN2",
        target_bir_lowering=False,
        debug=False,
        enable_asserts=False,
        num_devices=NCORES,
        name="biochem_x_stat_f8",
    )

    a_t = nc.dram_tensor(
        "a_t", [NG8 * KT, KQ8 * MSHARD], mybir.dt.float8e4,
        kind="ExternalInput").ap()
    xt8_d = nc.dram_tensor(
        "xt8", [KT, NKT2 * DIM], mybir.dt.float8e4,
        kind="ExternalInput").ap()
    xs_d = nc.dram_tensor("xs", [MT, NMT * DIM], bf16, kind="ExternalInput").ap()
    xst_d = nc.dram_tensor("xst", [DIM, MSHARD], bf16, kind="ExternalInput").ap()
    id_d = nc.dram_tensor("ident", [DIM, DIM], bf16, kind="ExternalInput").ap()
    # m-tile-major ([p, mt, d]) so the epilogue leaves in a few wide DMAs
    out_d = nc.dram_tensor("out", [MT, NMT * DIM], f32, kind="ExternalOutput").ap()
    with tile.TileContext(nc) as tc:
        with ExitStack() as ctx:
            _body_f8(ctx, tc, a_t, xt8_d, xs_d, xst_d, id_d, out_d)
    nc.compile()
    _nc_cache[key] = nc
    return nc


def prepare_in_maps(x, A, layout=None, mm_dtype=None):
    import ml_dtypes
    np_fp8 = np.dtype(ml_dtypes.float8_e4m3)
    np_bf16 = np.dtype(ml_dtypes.bfloat16)

    x = np.asarray(x, np.float32)
    A = np.asarray(A, np.float32)

    # x tiled into the [128, 79*64] stationary SBUF layout, fp8
    xp = np.zeros((KPAD2, DIM), np_fp8)
    xp[:N] = x.astype(np_fp8)
    xt8_np = np.ascontiguousarray(
        xp.reshape(NKT2, KT, DIM).transpose(1, 0, 2).reshape(KT, NKT2 * DIM)
    )
    ident = np.eye(DIM, dtype=np_bf16)

    A8 = A.astype(np_fp8)  # one 100 MB quantization pass, sliced per core

    in_maps = []
    for c in range(NCORES):
        sh = slice(c * MSHARD, (c + 1) * MSHARD)
        at8 = np.zeros((KPAD2, MSHARD), np_fp8)
        at8[:N] = A8[sh].T
        # slab layout: row gi*128+p, cols sub*1250:(sub+1)*1250 holds
        # at8[(k0+sub)*128 + p, :] for group gi=(k0, g)
        a_t_c = np.zeros((NG8 * KT, KQ8 * MSHARD), np_fp8)
        for gi, (k0, g) in enumerate(KGROUPS_F8):
            blk = at8[k0 * KT:(k0 + g) * KT, :]
            a_t_c[gi * KT:(gi + 1) * KT, :g * MSHARD] = (
                blk.reshape(g, KT, MSHARD).transpose(1, 0, 2).reshape(KT, g * MSHARD)
            )
        xs_c = np.ascontiguousarray(
            x[sh].reshape(NMT, MT, DIM).transpose(1, 0, 2)
            .reshape(MT, NMT * DIM).astype(np_bf16)
        )
        in_maps.append({
            "a_t": a_t_c,
            "xt8": xt8_np,
            "xs": xs_c,
            "xst": np.ascontiguousarray(x[sh].T.astype(np_bf16)),
            "ident": ident,
        })
    return in_maps


def run(inputs, trace=False, layout=None, mm_dtype=None, **spmd_kwargs):
    """Returns (full_output [10000, 64] f32, BassKernelResults)."""
    from concourse.bass_utils import run_bass_kernel_spmd

    nc = build()
    in_maps = prepare_in_maps(inputs["x"], inputs["A"])
    res = run_bass_kernel_spmd(
        nc, in_maps, core_ids=list(range(NCORES)), trace=trace, **spmd_kwargs
    )
    # undo the m-tile-major staging: [125, 10*64] -> [1250, 64]
    out = np.concatenate([
        res.results[c]["out"].reshape(MT, NMT, DIM)
        .transpose(1, 0, 2).reshape(MSHARD, DIM)
        for c in range(NCORES)
    ], axis=0)
    return out, res


def kernel(t=None, x=None, A=None):
    out, _ = run({"x": x, "A": A})
    return out


# revision 30
# speedup vs baseline: 1.0051x; 1.0051x over previous
"""Trainium2 Bass kernel for nn_BiochemicalDiffusion.

Computes  out = F - B*x - r * rowsum(x * (A @ x))  for A:[10000,10000] f32,
x:[10000,64] f32, across 8 NeuronCores.

Sharding (all done host-side in this file):
  - A is sharded row-wise: core c gets rows [c*1250, (c+1)*1250).
  - The shard is passed pre-transposed (A_shard^T, [10000, 1250]) so the PE
    can contract over k directly: Ax_shard = A_shard^T.T @ x.
  - x is passed in full to every core (it is tiny), pre-tiled into the
    [128, 79*64] SBUF layout the matmul consumes.
  - Each core computes its [1250, 64] slice of the output; the host
    concatenates them.

A is quantized host-side to fp8e4m3 (1 byte/elem; quantization errors are
zero-mean and average out over the 10000-long contraction -- measured
absmax-rel ~1.3e-3 vs the 2e-2 gate).  Matmuls run in DoubleRow perf mode:
both operands carry a pair of k-tiles ([128, 2, w] APs, contraction 256
per instruction).  PSUM chunk width 250 keeps a DR matmul at 1 column/
cycle ([128,2,500] drops to 2 cycles/col -- measured) and one f32 PSUM
bank per accumulator.

Scheduling notes (from ~14 traced structural experiments):
  - DMA completion semaphores fire up to ~8 us after their data lands
    mid-stream, and DMA issues gate on an 8-lane semaphore recycle
    (issue #k waits for #(k-8)'s sem).  The junk-heat delay line below
    absorbs that jitter; every attempt to remove it exposed the stalls
    directly on the in-order PE and lost 3-10 us.
  - PSUM accumulation groups must not share a PSUM bank.

Everything is hardcoded to the problem shapes; kernel.py is self-contained.
"""

import numpy as np

N = 10000
DIM = 64
NCORES = 8
MSHARD = N // NCORES  # 1250 rows of A / out per core
MT = 125              # m-tile (PSUM partition) size
NMT = MSHARD // MT    # 10 m-tiles per core
KT = 128              # k-tile (contraction) size

F_CONST = 1.0
B_CONST = 0.1
R_CONST = 0.01

NKT2 = 79                 # k-tiles (tile 78 is 16 real rows)
NPAIR = 39                # DoubleRow pairs (tiles 0..77)
KLAST = 16                # real rows in tile 78
KPAD2 = NKT2 * KT
# PSUM chunks: five 250-wide ([128,2,250] DR matmul = 1 column/cycle)
MCH5 = [(i * 250, (i + 1) * 250) for i in range(5)]
# DMA groups alternate between the sync and scalar HWDGE rings; each SDMA
# engine hides one ring's completion latency behind the other ring's data.
KQ8 = 8                   # max k-tiles per DMA group (1.28 MB transfers)
KGROUPS_F8 = ([(0, 2), (2, 2), (4, 4)]
              + [(8 + 8 * i, 8) for i in range(8)]
              + [(72, 4), (76, 2), (78, 1)])
NG8 = len(KGROUPS_F8)     # 14 groups covering all 79 tiles

_nc_cache = {}


def _body_f8(ctx, tc, a_t, xt8_d, xs_d, xst_d, id_d, out_d):
    """Pure-fp8 single pass with DoubleRow (contraction 256 per matmul).

    Baseline schedule (measured best of the structural variants): x k-tile
    pairs stationary, A^T slabs stream in groups alternating between the
    sync and scalar HWDGE rings; junk heat matmuls form a ~17 us delay
    line on the in-order PE that absorbs the DMA completion-semaphore
    jitter and keeps the PE clock-gate (HAM) at 2.4 GHz.

    Deltas vs the original baseline, all tail-local:
      - late per-group heat trimmed (gi >= 5): by then the real stream
        keeps the PE warm, and trailing junk only pushed the final
        matmuls + epilogue past the stream end.
      - epilogue constants in bf16 (240 KB vs 656 KB: land earlier,
        +~3e-4 rel err vs the 2e-2 gate).
      - epilogue: bf16 transposes (single-pass vs fp32 LOW_HIGH) and the
        two affine steps on the scalar (Activation) engine, pipelining
        the per-m-tile chain across PE/DVE/ACT."""
    import concourse.bass  # noqa: F401
    from concourse import mybir

    nc = tc.nc
    f32 = mybir.dt.float32
    fp8 = mybir.dt.float8e4
    bf16 = mybir.dt.bfloat16
    dr = mybir.MatmulPerfMode.DoubleRow

    consts = ctx.enter_context(tc.tile_pool(name="consts", bufs=1))
    slabs = ctx.enter_context(tc.tile_pool(name="slabs", bufs=6))
    psums = ctx.enter_context(tc.tile_pool(name="psums", bufs=1, space="PSUM"))
    ptp = ctx.enter_context(tc.tile_pool(name="ptp", bufs=2, space="PSUM"))
    epil = ctx.enter_context(tc.tile_pool(name="epil", bufs=2))

    # stationary x rides the scalar HWDGE ring ahead of the odd slab
    # groups; the epilogue-only constants are appended at the end of the
    # scalar program.
    xt8 = consts.tile([KT, NKT2, DIM], fp8)
    nc.scalar.dma_start(out=xt8[:, :8, :], in_=xt8_d[:, :8 * DIM])
    # epilogue-constant tiles (DMAs issued at gi==1 below) + flusher scratch
    xs = consts.tile([MT, NMT * DIM], bf16)
    xst = consts.tile([DIM, MSHARD], bf16)
    ident = consts.tile([DIM, DIM], bf16)
    scr = consts.tile([KT, 16], fp8)

    accs = [psums.tile([DIM, c1 - c0], f32, name=f"acc{i}", tag=f"acc{i}")
            for i, (c0, c1) in enumerate(MCH5)]

    # PE heat management: the HAM clock gate runs the PE at 1.2 GHz unless
    # it sees sustained busy (~3.4 us windows).  The junk matmuls (no DMA
    # deps; WAW-chained on one scratch tile) get hoisted by the scheduler
    # into a contiguous blob that delays the real stream just long enough
    # to ride out the early completion-sem stalls.
    junk_l = consts.tile([KT, 2, DIM], fp8)
    nc.vector.memset(junk_l, 0.5)
    junk_r = consts.tile([KT, 2, 250], fp8)
    nc.vector.memset(junk_r, 0.5)
    warm = psums.tile([DIM, 250], f32)

    def heat(n):
        for _ in range(n):
            nc.tensor.matmul(warm, lhsT=junk_l, rhs=junk_r,
                             start=True, stop=True, perf_mode=dr)

    heat(16)

    for gi, (k0, g) in enumerate(KGROUPS_F8):
        # alternate groups across the two HWDGE rings so each SDMA engine
        # hides one ring's completion latency behind the other's data
        dma_eng = nc.sync if gi % 2 == 0 else nc.scalar
        slab = slabs.tile([KT, KQ8, MSHARD], fp8, name=f"slab{gi}", tag="slab")
        if g == 1:  # trailing 16-row tile: only partitions 0:15 carry data
            dma_eng.dma_start(out=slab[:KLAST, :1, :],
                              in_=a_t[gi * KT:gi * KT + KLAST, :MSHARD])
            for i, (c0, c1) in enumerate(MCH5):
                nc.tensor.matmul(
                    accs[i],
                    lhsT=xt8[:KLAST, k0, :],
                    rhs=slab[:KLAST, 0, c0:c1],
                    start=False,
                    stop=True,
                )
            continue
        dma_eng.dma_start(out=slab[:, :g, :],
                          in_=a_t[gi * KT:(gi + 1) * KT, :g * MSHARD])
        if gi == 1:
            # the rest of the stationary x rides the scalar ring BEHIND
            # group 1 (whose data the stream needs first); tiles 8+ are
            # not consumed until pair 4, by which time this has landed
            nc.scalar.dma_start(out=xt8[:, 8:, :], in_=xt8_d[:, 8 * DIM:])
            # epilogue constants EARLY (land ~17 us) + one 2 KB flusher:
            # completion sems only fire when the ring's NEXT DMA finishes,
            # so the flusher fires ident's sem immediately instead of at
            # ring-idle (~49 us) -- that sem was gating the epilogue for
            # ~2 us after the last matmul.  The flusher also re-aligns
            # the 8-lane sem recycling onto early-firing sems.
            nc.scalar.dma_start(out=xs, in_=xs_d)
            nc.scalar.dma_start(out=xst, in_=xst_d)
            nc.scalar.dma_start(out=ident, in_=id_d)
            nc.scalar.dma_start(out=scr, in_=xt8_d[:, :16])
        for sub in range(0, g, 2):
            j = (k0 + sub) // 2  # pair index
            lhsT = xt8[:, k0 + sub:k0 + sub + 2, :]
            for i, (c0, c1) in enumerate(MCH5):
                nc.tensor.matmul(
                    accs[i],
                    lhsT=lhsT,
                    rhs=slab[:, sub:sub + 2, c0:c1],
                    start=(j == 0),
                    stop=False,
                    perf_mode=dr,
                )
        # per-group heat exactly as the measured-best baseline: the junk
        # delay line must cover the whole stream (trimming it after gi=4
        # exposed late-group sem stalls and lost ~5 us)
        heat(8 if gi <= 10 else 3)

    # P = x^T * Ax^T  (elementwise), [64, 1250] bf16 in SBUF (bf16
    # transposes run single-pass; fp32 runs LOW_HIGH two-pass)
    p_full = epil.tile([DIM, MSHARD], bf16, bufs=1)
    for i, (c0, c1) in enumerate(MCH5):
        nc.vector.tensor_mul(p_full[:, c0:c1], xst[:, c0:c1], accs[i])

    # out staged m-tile-major ([p, mt, d]); host restores row order.
    o_full = epil.tile([MT, NMT, DIM], f32, bufs=1)
    for mt in range(NMT):
        pt = ptp.tile([MT, DIM], bf16, name=f"pt{mt}", tag="pt")
        nc.tensor.transpose(
            out=pt, in_=p_full[:, mt * MT:(mt + 1) * MT], identity=ident,
        )
        s = epil.tile([MT, 1], f32, name=f"s{mt}", tag="s")
        nc.vector.tensor_reduce(
            out=s, in_=pt, axis=mybir.AxisListType.X, op=mybir.AluOpType.add,
        )
        t_col = epil.tile([MT, 1], f32, name=f"t{mt}", tag="t")
        # t = s * (-r) + F  and  o = x * (-b) + t on the Activation
        # engine -- the per-mt chain pipelines across PE/DVE/ACT
        nc.scalar.activation(
            out=t_col, in_=s, func=mybir.ActivationFunctionType.Identity,
            bias=F_CONST, scale=-R_CONST,
        )
        nc.scalar.activation(
            out=o_full[:, mt, :], in_=xs[:, mt * DIM:(mt + 1) * DIM],
            func=mybir.ActivationFunctionType.Identity,
            bias=t_col, scale=-B_CONST,
        )
        if mt == 3:
            # output leaves in three waves overlapping the epilogue chain;
            # the last wave is small so its transfer+receipt tail is short
            nc.scalar.dma_start(out=out_d[:, :4 * DIM], in_=o_full[:, :4, :])
        elif mt == 7:
            nc.sync.dma_start(out=out_d[:, 4 * DIM:8 * DIM],
                              in_=o_full[:, 4:8, :])
    nc.scalar.dma_start(out=out_d[:, 8 * DIM:], in_=o_full[:, 8:, :])


def build(layout=None, mm_dtype=None):
    key = "f8"
    if key in _nc_cache:
        return _nc_cache[key]

    from contextlib import ExitStack
    import concourse.tile as tile
    from concourse import bacc
    from concourse import mybir

    f32 = mybir.dt.float32
    bf16 = mybir.dt.bfloat16

    nc = bacc.Bacc(
        "TRN2",
        target_bir_lowering=False,
        debug=False,
        enable_asserts=False,
        num_devices=NCORES,
        name="biochem_x_stat_f8",
    )

    a_t = nc.dram_tensor(
        "a_t", [NG8 * KT, KQ8 * MSHARD], mybir.dt.float8e4,
        kind="ExternalInput").ap()
    xt8_d = nc.dram_tensor(
        "xt8", [KT, NKT2 * DIM], mybir.dt.float8e4,
        kind="ExternalInput").ap()
    xs_d = nc.dram_tensor("xs", [MT, NMT * DIM], bf16, kind="ExternalInput").ap()
    xst_d = nc.dram_tensor("xst", [DIM, MSHARD], bf16, kind="ExternalInput").ap()
    id_d = nc.dram_tensor("ident", [DIM, DIM], bf16, kind="ExternalInput").ap()
    # m-tile-major ([p, mt, d]) so the epilogue leaves in a few wide DMAs
    out_d = nc.dram_tensor("out", [MT, NMT * DIM], f32, kind="ExternalOutput").ap()
    with tile.TileContext(nc) as tc:
        with ExitStack() as ctx:
            _body_f8(ctx, tc, a_t, xt8_d, xs_d, xst_d, id_d, out_d)
    nc.compile()
    _nc_cache[key] = nc
    return nc


def prepare_in_maps(x, A, layout=None, mm_dtype=None):
    import ml_dtypes
    np_fp8 = np.dtype(ml_dtypes.float8_e4m3)
    np_bf16 = np.dtype(ml_dtypes.bfloat16)

    x = np.asarray(x, np.float32)
    A = np.asarray(A, np.float32)

    # x tiled into the [128, 79*64] stationary SBUF layout, fp8
    xp = np.zeros((KPAD2, DIM), np_fp8)
    xp[:N] = x.astype(np_fp8)
    xt8_np = np.ascontiguousarray(
        xp.reshape(NKT2, KT, DIM).transpose(1, 0, 2).reshape(KT, NKT2 * DIM)
    )
    ident = np.eye(DIM, dtype=np_bf16)

    A8 = A.astype(np_fp8)  # one 100 MB quantization pass, sliced per core

    in_maps = []
    for c in range(NCORES):
        sh = slice(c * MSHARD, (c + 1) * MSHARD)
        at8 = np.zeros((KPAD2, MSHARD), np_fp8)
        at8[:N] = A8[sh].T
        # slab layout: row gi*128+p, cols sub*1250:(sub+1)*1250 holds
        # at8[(k0+sub)*128 + p, :] for group gi=(k0, g)
        a_t_c = np.zeros((NG8 * KT, KQ8 * MSHARD), np_fp8)
        for gi, (k0, g) in enumerate(KGROUPS_F8):
            blk = at8[k0 * KT:(k0 + g) * KT, :]
            a_t_c[gi * KT:(gi + 1) * KT, :g * MSHARD] = (
                blk.reshape(g, KT, MSHARD).transpose(1, 0, 2).reshape(KT, g * MSHARD)
            )
        xs_c = np.ascontiguousarray(
            x[sh].reshape(NMT, MT, DIM).transpose(1, 0, 2)
            .reshape(MT, NMT * DIM).astype(np_bf16)
        )
        in_maps.append({
            "a_t": a_t_c,
            "xt8": xt8_np,
            "xs": xs_c,
            "xst": np.ascontiguousarray(x[sh].T.astype(np_bf16)),
            "ident": ident,
        })
    return in_maps


def run(inputs, trace=False, layout=None, mm_dtype=None, **spmd_kwargs):
    """Returns (full_output [10000, 64] f32, BassKernelResults)."""
    from concourse.bass_utils import run_bass_kernel_spmd

    nc = build()
    in_maps = prepare_in_maps(inputs["x"], inputs["A"])
    res = run_bass_kernel_spmd(
        nc, in_maps, core_ids=list(range(NCORES)), trace=trace, **spmd_kwargs
    )
    # undo the m-tile-major staging: [125, 10*64] -> [1250, 64]
    out = np.concatenate([
        res.results[c]["out"].reshape(MT, NMT, DIM)
        .transpose(1, 0, 2).reshape(MSHARD, DIM)
        for c in range(NCORES)
    ], axis=0)
    return out, res


def kernel(t=None, x=None, A=None):
    out, _ = run({"x": x, "A": A})
    return out


# revision 31
# speedup vs baseline: 1.0741x; 1.0686x over previous
"""Trainium2 Bass kernel for nn_BiochemicalDiffusion.

Computes  out = F - B*x - r * rowsum(x * (A @ x))  for A:[10000,10000] f32,
x:[10000,64] f32, across 8 NeuronCores.

Sharding (all done host-side in this file):
  - A is sharded row-wise: core c gets rows [c*1250, (c+1)*1250).
  - The shard is passed pre-transposed (A_shard^T, [10000, 1250]) so the PE
    can contract over k directly: Ax_shard = A_shard^T.T @ x.
  - x is passed in full to every core (it is tiny), pre-tiled into the
    [128, 79*64] SBUF layout the matmul consumes.
  - Each core computes its [1250, 64] slice of the output; the host
    concatenates them.

A is quantized host-side to fp8e4m3 (1 byte/elem; quantization errors are
zero-mean and average out over the 10000-long contraction -- measured
absmax-rel ~1.3e-3 vs the 2e-2 gate).  Matmuls run in DoubleRow perf mode:
both operands carry a pair of k-tiles ([128, 2, w] APs, contraction 256
per instruction).  PSUM chunk width 250 keeps a DR matmul at 1 column/
cycle ([128,2,500] drops to 2 cycles/col -- measured) and one f32 PSUM
bank per accumulator.

Scheduling notes (from ~14 traced structural experiments):
  - DMA completion semaphores fire up to ~8 us after their data lands
    mid-stream, and DMA issues gate on an 8-lane semaphore recycle
    (issue #k waits for #(k-8)'s sem).  The junk-heat delay line below
    absorbs that jitter; every attempt to remove it exposed the stalls
    directly on the in-order PE and lost 3-10 us.
  - PSUM accumulation groups must not share a PSUM bank.

Everything is hardcoded to the problem shapes; kernel.py is self-contained.
"""

import numpy as np

N = 10000
DIM = 64
NCORES = 8
MSHARD = N // NCORES  # 1250 rows of A / out per core
MT = 125              # m-tile (PSUM partition) size
NMT = MSHARD // MT    # 10 m-tiles per core
KT = 128              # k-tile (contraction) size

F_CONST = 1.0
B_CONST = 0.1
R_CONST = 0.01

NKT2 = 79                 # k-tiles (tile 78 is 16 real rows)
NPAIR = 39                # DoubleRow pairs (tiles 0..77)
KLAST = 16                # real rows in tile 78
KPAD2 = NKT2 * KT
# PSUM chunks: five 250-wide ([128,2,250] DR matmul = 1 column/cycle)
MCH5 = [(i * 250, (i + 1) * 250) for i in range(5)]
# DMA groups alternate between the sync and scalar HWDGE rings; each SDMA
# engine hides one ring's completion latency behind the other ring's data.
KQ8 = 8                   # max k-tiles per DMA group (1.28 MB transfers)
KGROUPS_F8 = ([(0, 2), (2, 2), (4, 4)]
              + [(8 + 8 * i, 8) for i in range(8)]
              + [(72, 4), (76, 2), (78, 1)])
NG8 = len(KGROUPS_F8)     # 14 groups covering all 79 tiles

_nc_cache = {}


def _body_f8(ctx, tc, a_t, xt8_d, xs_d, xst_d, id_d, out_d):
    """Pure-fp8 single pass with DoubleRow (contraction 256 per matmul).

    Baseline schedule (measured best of the structural variants): x k-tile
    pairs stationary, A^T slabs stream in groups alternating between the
    sync and scalar HWDGE rings; junk heat matmuls form a ~17 us delay
    line on the in-order PE that absorbs the DMA completion-semaphore
    jitter and keeps the PE clock-gate (HAM) at 2.4 GHz.

    Deltas vs the original baseline, all tail-local:
      - late per-group heat trimmed (gi >= 5): by then the real stream
        keeps the PE warm, and trailing junk only pushed the final
        matmuls + epilogue past the stream end.
      - epilogue constants in bf16 (240 KB vs 656 KB: land earlier,
        +~3e-4 rel err vs the 2e-2 gate).
      - epilogue: bf16 transposes (single-pass vs fp32 LOW_HIGH) and the
        two affine steps on the scalar (Activation) engine, pipelining
        the per-m-tile chain across PE/DVE/ACT."""
    import concourse.bass  # noqa: F401
    from concourse import mybir

    nc = tc.nc
    f32 = mybir.dt.float32
    fp8 = mybir.dt.float8e4
    bf16 = mybir.dt.bfloat16
    dr = mybir.MatmulPerfMode.DoubleRow

    consts = ctx.enter_context(tc.tile_pool(name="consts", bufs=1))
    slabs = ctx.enter_context(tc.tile_pool(name="slabs", bufs=6))
    psums = ctx.enter_context(tc.tile_pool(name="psums", bufs=1, space="PSUM"))
    ptp = ctx.enter_context(tc.tile_pool(name="ptp", bufs=2, space="PSUM"))
    epil = ctx.enter_context(tc.tile_pool(name="epil", bufs=2))

    # stationary x rides the scalar HWDGE ring ahead of the odd slab
    # groups; the epilogue-only constants are appended at the end of the
    # scalar program.
    xt8 = consts.tile([KT, NKT2, DIM], fp8)
    nc.scalar.dma_start(out=xt8[:, :8, :], in_=xt8_d[:, :8 * DIM])

    accs = [psums.tile([DIM, c1 - c0], f32, name=f"acc{i}", tag=f"acc{i}")
            for i, (c0, c1) in enumerate(MCH5)]

    # PE heat management: the HAM clock gate runs the PE at 1.2 GHz unless
    # it sees sustained busy (~3.4 us windows).  The junk matmuls (no DMA
    # deps; WAW-chained on one scratch tile) get hoisted by the scheduler
    # into a contiguous blob that delays the real stream just long enough
    # to ride out the early completion-sem stalls.
    junk_l = consts.tile([KT, 2, DIM], fp8)
    nc.vector.memset(junk_l, 0.5)
    junk_r = consts.tile([KT, 2, 250], fp8)
    nc.vector.memset(junk_r, 0.5)
    warm = psums.tile([DIM, 250], f32)

    def heat(n):
        for _ in range(n):
            nc.tensor.matmul(warm, lhsT=junk_l, rhs=junk_r,
                             start=True, stop=True, perf_mode=dr)

    heat(16)

    for gi, (k0, g) in enumerate(KGROUPS_F8):
        # alternate groups across the two HWDGE rings so each SDMA engine
        # hides one ring's completion latency behind the other's data
        dma_eng = nc.sync if gi % 2 == 0 else nc.scalar
        slab = slabs.tile([KT, KQ8, MSHARD], fp8, name=f"slab{gi}", tag="slab")
        if g == 1:  # trailing 16-row tile: only partitions 0:15 carry data
            dma_eng.dma_start(out=slab[:KLAST, :1, :],
                              in_=a_t[gi * KT:gi * KT + KLAST, :MSHARD])
            for i, (c0, c1) in enumerate(MCH5):
                nc.tensor.matmul(
                    accs[i],
                    lhsT=xt8[:KLAST, k0, :],
                    rhs=slab[:KLAST, 0, c0:c1],
                    start=False,
                    stop=True,
                )
            continue
        dma_eng.dma_start(out=slab[:, :g, :],
                          in_=a_t[gi * KT:(gi + 1) * KT, :g * MSHARD])
        if gi == 1:
            # the rest of the stationary x rides the scalar ring BEHIND
            # group 1 (whose data the stream needs first); tiles 8+ are
            # not consumed until pair 4, by which time this has landed
            nc.scalar.dma_start(out=xt8[:, 8:, :], in_=xt8_d[:, 8 * DIM:])
        for sub in range(0, g, 2):
            j = (k0 + sub) // 2  # pair index
            lhsT = xt8[:, k0 + sub:k0 + sub + 2, :]
            for i, (c0, c1) in enumerate(MCH5):
                nc.tensor.matmul(
                    accs[i],
                    lhsT=lhsT,
                    rhs=slab[:, sub:sub + 2, c0:c1],
                    start=(j == 0),
                    stop=False,
                    perf_mode=dr,
                )
        # per-group heat exactly as the measured-best baseline: the junk
        # delay line must cover the whole stream (trimming it after gi=4
        # exposed late-group sem stalls and lost ~5 us)
        heat(8 if gi <= 10 else 3)

    # epilogue-only constants (bf16), appended behind the odd slab stream
    xs = consts.tile([MT, NMT * DIM], bf16)
    nc.scalar.dma_start(out=xs, in_=xs_d)
    xst = consts.tile([DIM, MSHARD], bf16)
    nc.scalar.dma_start(out=xst, in_=xst_d)
    ident = consts.tile([DIM, DIM], bf16)
    nc.scalar.dma_start(out=ident, in_=id_d)

    # P = x^T * Ax^T  (elementwise), [64, 1250] bf16 in SBUF (bf16
    # transposes run single-pass; fp32 runs LOW_HIGH two-pass)
    p_full = epil.tile([DIM, MSHARD], bf16, bufs=1)
    for i, (c0, c1) in enumerate(MCH5):
        nc.vector.tensor_mul(p_full[:, c0:c1], xst[:, c0:c1], accs[i])

    # out staged m-tile-major ([p, mt, d]); host restores row order.
    o_full = epil.tile([MT, NMT, DIM], f32, bufs=1)
    for mt in range(NMT):
        pt = ptp.tile([MT, DIM], bf16, name=f"pt{mt}", tag="pt")
        nc.tensor.transpose(
            out=pt, in_=p_full[:, mt * MT:(mt + 1) * MT], identity=ident,
        )
        s = epil.tile([MT, 1], f32, name=f"s{mt}", tag="s")
        nc.vector.tensor_reduce(
            out=s, in_=pt, axis=mybir.AxisListType.X, op=mybir.AluOpType.add,
        )
        t_col = epil.tile([MT, 1], f32, name=f"t{mt}", tag="t")
        # t = s * (-r) + F  and  o = x * (-b) + t on the Activation
        # engine -- the per-mt chain pipelines across PE/DVE/ACT
        nc.scalar.activation(
            out=t_col, in_=s, func=mybir.ActivationFunctionType.Identity,
            bias=F_CONST, scale=-R_CONST,
        )
        nc.scalar.activation(
            out=o_full[:, mt, :], in_=xs[:, mt * DIM:(mt + 1) * DIM],
            func=mybir.ActivationFunctionType.Identity,
            bias=t_col, scale=-B_CONST,
        )
        if mt == 3:
            # output leaves in three waves overlapping the epilogue chain;
            # the last wave is small so its transfer+receipt tail is short
            nc.scalar.dma_start(out=out_d[:, :4 * DIM], in_=o_full[:, :4, :])
        elif mt == 7:
            nc.sync.dma_start(out=out_d[:, 4 * DIM:8 * DIM],
                              in_=o_full[:, 4:8, :])
    nc.scalar.dma_start(out=out_d[:, 8 * DIM:], in_=o_full[:, 8:, :])


def build(layout=None, mm_dtype=None):
    key = "f8"
    if key in _nc_cache:
        return _nc_cache[key]

    from contextlib import ExitStack
    import concourse.tile as tile
    from concourse import bacc
    from concourse import mybir

    f32 = mybir.dt.float32
    bf16 = mybir.dt.bfloat16

    nc = bacc.Bacc(
        "TRN2",
        target_bir_lowering=False,
        debug=False,
        enable_asserts=False,
        num_devices=NCORES,
        name="biochem_x_stat_f8",
    )

    a_t = nc.dram_tensor(
        "a_t", [NG8 * KT, KQ8 * MSHARD], mybir.dt.float8e4,
        kind="ExternalInput").ap()
    xt8_d = nc.dram_tensor(
        "xt8", [KT, NKT2 * DIM], mybir.dt.float8e4,
        kind="ExternalInput").ap()
    xs_d = nc.dram_tensor("xs", [MT, NMT * DIM], bf16, kind="ExternalInput").ap()
    xst_d = nc.dram_tensor("xst", [DIM, MSHARD], bf16, kind="ExternalInput").ap()
    id_d = nc.dram_tensor("ident", [DIM, DIM], bf16, kind="ExternalInput").ap()
    # m-tile-major ([p, mt, d]) so the epilogue leaves in a few wide DMAs
    out_d = nc.dram_tensor("out", [MT, NMT * DIM], f32, kind="ExternalOutput").ap()
    with tile.TileContext(nc) as tc:
        with ExitStack() as ctx:
            _body_f8(ctx, tc, a_t, xt8_d, xs_d, xst_d, id_d, out_d)
    nc.compile()
    _nc_cache[key] = nc
    return nc


def prepare_in_maps(x, A, layout=None, mm_dtype=None):
    import ml_dtypes
    np_fp8 = np.dtype(ml_dtypes.float8_e4m3)
    np_bf16 = np.dtype(ml_dtypes.bfloat16)

    x = np.asarray(x, np.float32)
    A = np.asarray(A, np.float32)

    # x tiled into the [128, 79*64] stationary SBUF layout, fp8
    xp = np.zeros((KPAD2, DIM), np_fp8)
    xp[:N] = x.astype(np_fp8)
    xt8_np = np.ascontiguousarray(
        xp.reshape(NKT2, KT, DIM).transpose(1, 0, 2).reshape(KT, NKT2 * DIM)
    )
    ident = np.eye(DIM, dtype=np_bf16)

    A8 = A.astype(np_fp8)  # one 100 MB quantization pass, sliced per core

    in_maps = []
    for c in range(NCORES):
        sh = slice(c * MSHARD, (c + 1) * MSHARD)
        at8 = np.zeros((KPAD2, MSHARD), np_fp8)
        at8[:N] = A8[sh].T
        # slab layout: row gi*128+p, cols sub*1250:(sub+1)*1250 holds
        # at8[(k0+sub)*128 + p, :] for group gi=(k0, g)
        a_t_c = np.zeros((NG8 * KT, KQ8 * MSHARD), np_fp8)
        for gi, (k0, g) in enumerate(KGROUPS_F8):
            blk = at8[k0 * KT:(k0 + g) * KT, :]
            a_t_c[gi * KT:(gi + 1) * KT, :g * MSHARD] = (
                blk.reshape(g, KT, MSHARD).transpose(1, 0, 2).reshape(KT, g * MSHARD)
            )
        xs_c = np.ascontiguousarray(
            x[sh].reshape(NMT, MT, DIM).transpose(1, 0, 2)
            .reshape(MT, NMT * DIM).astype(np_bf16)
        )
        in_maps.append({
            "a_t": a_t_c,
            "xt8": xt8_np,
            "xs": xs_c,
            "xst": np.ascontiguousarray(x[sh].T.astype(np_bf16)),
            "ident": ident,
        })
    return in_maps


def run(inputs, trace=False, layout=None, mm_dtype=None, **spmd_kwargs):
    """Returns (full_output [10000, 64] f32, BassKernelResults)."""
    from concourse.bass_utils import run_bass_kernel_spmd

    nc = build()
    in_maps = prepare_in_maps(inputs["x"], inputs["A"])
    res = run_bass_kernel_spmd(
        nc, in_maps, core_ids=list(range(NCORES)), trace=trace, **spmd_kwargs
    )
    # undo the m-tile-major staging: [125, 10*64] -> [1250, 64]
    out = np.concatenate([
        res.results[c]["out"].reshape(MT, NMT, DIM)
        .transpose(1, 0, 2).reshape(MSHARD, DIM)
        for c in range(NCORES)
    ], axis=0)
    return out, res


def kernel(t=None, x=None, A=None):
    out, _ = run({"x": x, "A": A})
    return out


# revision 33
# speedup vs baseline: 1.0957x; 1.0201x over previous
"""Trainium2 Bass kernel for nn_BiochemicalDiffusion.

Computes  out = F - B*x - r * rowsum(x * (A @ x))  for A:[10000,10000] f32,
x:[10000,64] f32, across 8 NeuronCores.

Sharding (all done host-side in this file):
  - A is sharded row-wise: core c gets rows [c*1250, (c+1)*1250).
  - The shard is passed pre-transposed (A_shard^T, [10000, 1250]) so the PE
    can contract over k directly: Ax_shard = A_shard^T.T @ x.
  - x is passed in full to every core (it is tiny), pre-tiled into the
    [128, 79*64] SBUF layout the matmul consumes.
  - Each core computes its [1250, 64] slice of the output; the host
    concatenates them.

A is quantized host-side to fp8e4m3 (1 byte/elem; quantization errors are
zero-mean and average out over the 10000-long contraction -- measured
absmax-rel ~1.3e-3 vs the 2e-2 gate).  Matmuls run in DoubleRow perf mode:
both operands carry a pair of k-tiles ([128, 2, w] APs, contraction 256
per instruction).  PSUM chunk width 250 keeps a DR matmul at 1 column/
cycle ([128,2,500] drops to 2 cycles/col -- measured) and one f32 PSUM
bank per accumulator.

Scheduling notes (from ~14 traced structural experiments):
  - DMA completion semaphores fire up to ~8 us after their data lands
    mid-stream, and DMA issues gate on an 8-lane semaphore recycle
    (issue #k waits for #(k-8)'s sem).  The junk-heat delay line below
    absorbs that jitter; every attempt to remove it exposed the stalls
    directly on the in-order PE and lost 3-10 us.
  - PSUM accumulation groups must not share a PSUM bank.

Everything is hardcoded to the problem shapes; kernel.py is self-contained.
"""

import numpy as np

N = 10000
DIM = 64
NCORES = 8
MSHARD = N // NCORES  # 1250 rows of A / out per core
MT = 125              # m-tile (PSUM partition) size
NMT = MSHARD // MT    # 10 m-tiles per core
KT = 128              # k-tile (contraction) size

F_CONST = 1.0
B_CONST = 0.1
R_CONST = 0.01

NKT2 = 79                 # k-tiles (tile 78 is 16 real rows)
NPAIR = 39                # DoubleRow pairs (tiles 0..77)
KLAST = 16                # real rows in tile 78
KPAD2 = NKT2 * KT
# PSUM chunks: five 250-wide ([128,2,250] DR matmul = 1 column/cycle)
MCH5 = [(i * 250, (i + 1) * 250) for i in range(5)]
# DMA groups alternate between the sync and scalar HWDGE rings; each SDMA
# engine hides one ring's completion latency behind the other ring's data.
KQ8 = 8                   # max k-tiles per DMA group (1.28 MB transfers)
KGROUPS_F8 = ([(0, 2), (2, 2), (4, 4)]
              + [(8 + 8 * i, 8) for i in range(8)]
              + [(72, 4), (76, 2), (78, 1)])
NG8 = len(KGROUPS_F8)     # 14 groups covering all 79 tiles

_nc_cache = {}


def _body_f8(ctx, tc, a_t, xt8_d, xs_d, xst_d, id_d, out_d):
    """Pure-fp8 single pass with DoubleRow (contraction 256 per matmul).

    Baseline schedule (measured best of the structural variants): x k-tile
    pairs stationary, A^T slabs stream in groups alternating between the
    sync and scalar HWDGE rings; junk heat matmuls form a ~17 us delay
    line on the in-order PE that absorbs the DMA completion-semaphore
    jitter and keeps the PE clock-gate (HAM) at 2.4 GHz.

    Deltas vs the original baseline, all tail-local (the heat schedule
    and DMA order are untouched -- trimming heat or moving the consts
    earlier both measured WORSE by 4-5 us):
      - epilogue constants in bf16 (240 KB vs 656 KB: land earlier,
        +~3e-4 rel err vs the 2e-2 gate).
      - epilogue: bf16 transposes (single-pass vs fp32 LOW_HIGH) and the
        two affine steps on the scalar (Activation) engine, pipelining
        the per-m-tile chain across PE/DVE/ACT."""
    import concourse.bass  # noqa: F401
    from concourse import mybir

    nc = tc.nc
    f32 = mybir.dt.float32
    fp8 = mybir.dt.float8e4
    bf16 = mybir.dt.bfloat16
    dr = mybir.MatmulPerfMode.DoubleRow

    consts = ctx.enter_context(tc.tile_pool(name="consts", bufs=1))
    slabs = ctx.enter_context(tc.tile_pool(name="slabs", bufs=6))
    psums = ctx.enter_context(tc.tile_pool(name="psums", bufs=1, space="PSUM"))
    ptp = ctx.enter_context(tc.tile_pool(name="ptp", bufs=2, space="PSUM"))
    epil = ctx.enter_context(tc.tile_pool(name="epil", bufs=2))

    # stationary x rides the scalar HWDGE ring ahead of the odd slab
    # groups; the epilogue-only constants are appended at the end of the
    # scalar program.
    xt8 = consts.tile([KT, NKT2, DIM], fp8)
    nc.scalar.dma_start(out=xt8[:, :8, :], in_=xt8_d[:, :8 * DIM])

    accs = [psums.tile([DIM, c1 - c0], f32, name=f"acc{i}", tag=f"acc{i}")
            for i, (c0, c1) in enumerate(MCH5)]

    # PE heat management: the HAM clock gate runs the PE at 1.2 GHz unless
    # it sees sustained busy (~3.4 us windows).  The junk matmuls (no DMA
    # deps; WAW-chained on one scratch tile) get hoisted by the scheduler
    # into a contiguous blob that delays the real stream just long enough
    # to ride out the early completion-sem stalls.
    junk_l = consts.tile([KT, 2, DIM], fp8)
    nc.vector.memset(junk_l, 0.5)
    junk_r = consts.tile([KT, 2, 250], fp8)
    nc.vector.memset(junk_r, 0.5)
    warm = psums.tile([DIM, 250], f32)

    def heat(n):
        for _ in range(n):
            nc.tensor.matmul(warm, lhsT=junk_l, rhs=junk_r,
                             start=True, stop=True, perf_mode=dr)

    heat(16)

    for gi, (k0, g) in enumerate(KGROUPS_F8):
        # alternate groups across the two HWDGE rings so each SDMA engine
        # hides one ring's completion latency behind the other's data
        dma_eng = nc.sync if gi % 2 == 0 else nc.scalar
        slab = slabs.tile([KT, KQ8, MSHARD], fp8, name=f"slab{gi}", tag="slab")
        if g == 1:  # trailing 16-row tile: only partitions 0:15 carry data
            dma_eng.dma_start(out=slab[:KLAST, :1, :],
                              in_=a_t[gi * KT:gi * KT + KLAST, :MSHARD])
            for i, (c0, c1) in enumerate(MCH5):
                nc.tensor.matmul(
                    accs[i],
                    lhsT=xt8[:KLAST, k0, :],
                    rhs=slab[:KLAST, 0, c0:c1],
                    start=False,
                    stop=True,
                )
            continue
        dma_eng.dma_start(out=slab[:, :g, :],
                          in_=a_t[gi * KT:(gi + 1) * KT, :g * MSHARD])
        if gi == 1:
            # the rest of the stationary x rides the scalar ring BEHIND
            # group 1 (whose data the stream needs first); tiles 8+ are
            # not consumed until pair 4, by which time this has landed
            nc.scalar.dma_start(out=xt8[:, 8:, :], in_=xt8_d[:, 8 * DIM:])
        for sub in range(0, g, 2):
            j = (k0 + sub) // 2  # pair index
            lhsT = xt8[:, k0 + sub:k0 + sub + 2, :]
            for i, (c0, c1) in enumerate(MCH5):
                nc.tensor.matmul(
                    accs[i],
                    lhsT=lhsT,
                    rhs=slab[:, sub:sub + 2, c0:c1],
                    start=(j == 0),
                    stop=False,
                    perf_mode=dr,
                )
        # per-group heat exactly as the measured-best baseline: the junk
        # delay line must cover the whole stream (trimming it after gi=4
        # exposed late-group sem stalls and lost ~5 us)
        heat(8 if gi <= 10 else 3)

    # epilogue-only constants (bf16), appended behind the odd slab stream
    xs = consts.tile([MT, NMT * DIM], bf16)
    nc.scalar.dma_start(out=xs, in_=xs_d)
    xst = consts.tile([DIM, MSHARD], bf16)
    nc.scalar.dma_start(out=xst, in_=xst_d)
    ident = consts.tile([DIM, DIM], bf16)
    nc.scalar.dma_start(out=ident, in_=id_d)
    # 2 KB flusher appended at the ring tail: completion sems only fire
    # when the ring's NEXT DMA finishes, so this fires ident's sem at
    # ~47 us instead of ring-idle+2.4 (~49.3 us), un-gating the epilogue
    # ~2 us earlier.  Touches nothing in the A-stream (pure append).
    scr = consts.tile([KT, 16], fp8)
    nc.scalar.dma_start(out=scr, in_=xt8_d[:, :16])

    # P = x^T * Ax^T  (elementwise), [64, 1250] bf16 in SBUF (bf16
    # transposes run single-pass; fp32 runs LOW_HIGH two-pass)
    p_full = epil.tile([DIM, MSHARD], bf16, bufs=1)
    for i, (c0, c1) in enumerate(MCH5):
        nc.vector.tensor_mul(p_full[:, c0:c1], xst[:, c0:c1], accs[i])

    # out staged m-tile-major ([p, mt, d]); host restores row order.
    o_full = epil.tile([MT, NMT, DIM], f32, bufs=1)
    for mt in range(NMT):
        pt = ptp.tile([MT, DIM], bf16, name=f"pt{mt}", tag="pt")
        nc.tensor.transpose(
            out=pt, in_=p_full[:, mt * MT:(mt + 1) * MT], identity=ident,
        )
        s = epil.tile([MT, 1], f32, name=f"s{mt}", tag="s")
        nc.vector.tensor_reduce(
            out=s, in_=pt, axis=mybir.AxisListType.X, op=mybir.AluOpType.add,
        )
        t_col = epil.tile([MT, 1], f32, name=f"t{mt}", tag="t")
        # t = s * (-r) + F  and  o = x * (-b) + t on the Activation
        # engine -- the per-mt chain pipelines across PE/DVE/ACT
        nc.scalar.activation(
            out=t_col, in_=s, func=mybir.ActivationFunctionType.Identity,
            bias=F_CONST, scale=-R_CONST,
        )
        nc.scalar.activation(
            out=o_full[:, mt, :], in_=xs[:, mt * DIM:(mt + 1) * DIM],
            func=mybir.ActivationFunctionType.Identity,
            bias=t_col, scale=-B_CONST,
        )
        if mt == 3:
            # output leaves in three waves overlapping the epilogue chain;
            # the last wave is small so its transfer+receipt tail is short
            nc.scalar.dma_start(out=out_d[:, :4 * DIM], in_=o_full[:, :4, :])
        elif mt == 7:
            nc.sync.dma_start(out=out_d[:, 4 * DIM:8 * DIM],
                              in_=o_full[:, 4:8, :])
    nc.scalar.dma_start(out=out_d[:, 8 * DIM:], in_=o_full[:, 8:, :])


def build(layout=None, mm_dtype=None):
    key = "f8"
    if key in _nc_cache:
        return _nc_cache[key]

    from contextlib import ExitStack
    import concourse.tile as tile
    from concourse import bacc
    from concourse import mybir

    f32 = mybir.dt.float32
    bf16 = mybir.dt.bfloat16

    nc = bacc.Bacc(
        "TRN2",
        target_bir_lowering=False,
        debug=False,
        enable_asserts=False,
        num_devices=NCORES,
        name="biochem_x_stat_f8",
    )

    a_t = nc.dram_tensor(
        "a_t", [NG8 * KT, KQ8 * MSHARD], mybir.dt.float8e4,
        kind="ExternalInput").ap()
    xt8_d = nc.dram_tensor(
        "xt8", [KT, NKT2 * DIM], mybir.dt.float8e4,
        kind="ExternalInput").ap()
    xs_d = nc.dram_tensor("xs", [MT, NMT * DIM], bf16, kind="ExternalInput").ap()
    xst_d = nc.dram_tensor("xst", [DIM, MSHARD], bf16, kind="ExternalInput").ap()
    id_d = nc.dram_tensor("ident", [DIM, DIM], bf16, kind="ExternalInput").ap()
    # m-tile-major ([p, mt, d]) so the epilogue leaves in a few wide DMAs
    out_d = nc.dram_tensor("out", [MT, NMT * DIM], f32, kind="ExternalOutput").ap()
    with tile.TileContext(nc) as tc:
        with ExitStack() as ctx:
            _body_f8(ctx, tc, a_t, xt8_d, xs_d, xst_d, id_d, out_d)
    nc.compile()
    _nc_cache[key] = nc
    return nc


def prepare_in_maps(x, A, layout=None, mm_dtype=None):
    import ml_dtypes
    np_fp8 = np.dtype(ml_dtypes.float8_e4m3)
    np_bf16 = np.dtype(ml_dtypes.bfloat16)

    x = np.asarray(x, np.float32)
    A = np.asarray(A, np.float32)

    # x tiled into the [128, 79*64] stationary SBUF layout, fp8
    xp = np.zeros((KPAD2, DIM), np_fp8)
    xp[:N] = x.astype(np_fp8)
    xt8_np = np.ascontiguousarray(
        xp.reshape(NKT2, KT, DIM).transpose(1, 0, 2).reshape(KT, NKT2 * DIM)
    )
    ident = np.eye(DIM, dtype=np_bf16)

    A8 = A.astype(np_fp8)  # one 100 MB quantization pass, sliced per core

    in_maps = []
    for c in range(NCORES):
        sh = slice(c * MSHARD, (c + 1) * MSHARD)
        at8 = np.zeros((KPAD2, MSHARD), np_fp8)
        at8[:N] = A8[sh].T
        # slab layout: row gi*128+p, cols sub*1250:(sub+1)*1250 holds
        # at8[(k0+sub)*128 + p, :] for group gi=(k0, g)
        a_t_c = np.zeros((NG8 * KT, KQ8 * MSHARD), np_fp8)
        for gi, (k0, g) in enumerate(KGROUPS_F8):
            blk = at8[k0 * KT:(k0 + g) * KT, :]
            a_t_c[gi * KT:(gi + 1) * KT, :g * MSHARD] = (
                blk.reshape(g, KT, MSHARD).transpose(1, 0, 2).reshape(KT, g * MSHARD)
            )
        xs_c = np.ascontiguousarray(
            x[sh].reshape(NMT, MT, DIM).transpose(1, 0, 2)
            .reshape(MT, NMT * DIM).astype(np_bf16)
        )
        in_maps.append({
            "a_t": a_t_c,
            "xt8": xt8_np,
            "xs": xs_c,
            "xst": np.ascontiguousarray(x[sh].T.astype(np_bf16)),
            "ident": ident,
        })
    return in_maps


def run(inputs, trace=False, layout=None, mm_dtype=None, **spmd_kwargs):
    """Returns (full_output [10000, 64] f32, BassKernelResults)."""
    from concourse.bass_utils import run_bass_kernel_spmd

    nc = build()
    in_maps = prepare_in_maps(inputs["x"], inputs["A"])
    res = run_bass_kernel_spmd(
        nc, in_maps, core_ids=list(range(NCORES)), trace=trace, **spmd_kwargs
    )
    # undo the m-tile-major staging: [125, 10*64] -> [1250, 64]
    out = np.concatenate([
        res.results[c]["out"].reshape(MT, NMT, DIM)
        .transpose(1, 0, 2).reshape(MSHARD, DIM)
        for c in range(NCORES)
    ], axis=0)
    return out, res


def kernel(t=None, x=None, A=None):
    out, _ = run({"x": x, "A": A})
    return out
